# revision 1
# baseline (speedup 1.0000x reference)
"""Bilateral filter (nn_BilateralFilter) Trainium2 Bass kernel.

Semantics (KERNEL_SIZE=5, THETA_ALPHA=2.0, THETA_BETA=0.1):
    w_k   = exp(-(dx^2+dy^2)/8)                      (24 offsets, center dropped)
    Ki    = exp(-50*(I(p+k) - I(p))^2)               per image channel c
    out[c,n,p] = sum_k w_k*Ki[c,k,p]*Q(n,p+k) / sum_k w_k*Ki[c,k,p]

Sharding: 8 cores = 2 batches x 4 col-slabs of 80 output cols.  Per core,
partitions = 128 = (xh in {0,1} col-half of 40) x (row mod 64); free dims =
(row-chunk q in [0,5), channel, x).  H=320 = 5*64 exactly, so every compute
op runs on 128 fully-occupied partitions (the 199us v1 used only 80).

All on-chip layouts are c-major so (c,q) merges into one AP dim:
- Ia[(s, c, q, xi44)]; Q replicated x3 over c as Qa[(s, c, q, n, xi44)]
  (host pre-gathers both, incl. the 5 dr-shifted copies, so input DMAs are
  contiguous per-dr slices spread across the sync/gpsimd/scalar queues)
- d/kw[(slot, c, q, x40)], P5[(j, c, q, n, x40)], acc/out[(c, q, n, x40)]
Products run as ONE 3600-elem DVE op per slot ((cq)15, n6-bcast, x40) - 24
ops total; dc/dr-folds are big flat fp16 adds; the norm fold tree emits
(c,q,x) directly and is split so only its last 3000 elems depend on the
final dr block, with the reciprocal chain interleaved between the last
products; final division is 3 per-channel muls + DMAs on separate queues.

Engine split (measured): DVE owns every tensor-tensor op at the fp16 2x
rate (~0.50 ns/elem/partition + ~175 ns/op; AP striding/broadcast do not
change DVE op cost).  GpSimd is compute-idle on purpose: concurrent
GpSimd+DVE execution collapses DVE throughput ~4x (SBUF contention), which
makes any offload net-negative.  ACT does Square + per-slot Exp
(bias = SHIFT + ln w_k; SHIFT=8 keeps fp16 sums in range and cancels in
the division) and the f16<->f32 reciprocal casts, fully overlapped.
GpSimd runs only the startup memsets and one late input DMA (before /
outside the DVE stream).
dr blocks are pipelined: sub -> square -> exp -> products -> fold.
Measured: ~126.5 us on HW (8 cores, max core), rel err 6.6e-4 vs fp32 ref.
"""

import math

import numpy as np

B, C, NCL = 2, 3, 6
H = W = 320
KS, PAD = 5, 2
SHIFT = 8.0
COEF = 50.0
XSL = W // 4              # 80 output cols per core slab
XWO = 40                  # output cols per half
XWI = XWO + 2 * PAD       # 44 input cols per half
NQ = 5                    # row chunks of 64
PR = 128
HP = H + 2 * PAD          # 324 padded rows

IW = NQ * XWI             # 220   Ia per (s,c)
FW_IA = KS * C * IW       # 3300
QB = NCL * XWI            # 264   Qa per (s,c,q)
FW_QA = KS * C * NQ * QB  # 19800 (Q replicated x3 over c)
SLW = C * NQ * XWO        # 600   d/kw per slot (c,q,x)
NWX = NQ * NCL * XWO      # 1200  per-c product block (q,n,x)
CQN = C * NWX             # 3600  per-j product block (c,q,n,x)
FW_D = KS * KS * SLW      # 15000
FW_N = C * NQ * XWO       # 600   norm (c,q,x)

_CACHE: dict = {}


def _emit(tc, i_ap, q_ap, out_ap):
    import concourse.bass as bass
    import concourse.mybir as mybir

    f16 = mybir.dt.float16
    f32 = mybir.dt.float32
    AF = mybir.ActivationFunctionType
    nc = tc.nc

    wy = [math.exp(-((s - PAD) ** 2) / 8.0) for s in range(KS)]

    def ap(t, off, dims):
        return bass.AP(tensor=t.tensor, offset=t.offset + off, ap=[[t.shape[1], PR]] + dims)

    with tc.tile_pool(name="p", bufs=1) as pool:
        Ia = pool.tile([PR, FW_IA], f16, tag="Ia")
        Qa = pool.tile([PR, FW_QA], f16, tag="Qa")
        d = pool.tile([PR, FW_D], f16, tag="d")
        kw = pool.tile([PR, FW_D], f16, tag="kw")
        P5 = pool.tile([PR, KS * CQN], f16, tag="P5")
        acc = pool.tile([PR, CQN], f16, tag="acc")
        s1 = pool.tile([PR, 2 * CQN], f16, tag="s1")
        ot = pool.tile([PR, CQN], f16, tag="out")

        nA10 = pool.tile([PR, 10 * SLW], f16, tag="nA10")
        normA = pool.tile([PR, FW_N], f16, tag="normA")
        nB2 = pool.tile([PR, 2 * SLW], f16, tag="nB2")
        norm = pool.tile([PR, FW_N], f16, tag="norm")
        n32 = pool.tile([PR, FW_N], f32, tag="n32")
        r32 = pool.tile([PR, FW_N], f32, tag="r32")
        rnh = pool.tile([PR, FW_N], f16, tag="rnh")

        # per-slot exp biases SHIFT + ln(w_k) as const columns (5 distinct)
        bias_vals = sorted(
            {
                SHIFT + math.log(wy[s] * wy[dc])
                for s in range(KS)
                for dc in range(KS)
                if not (s == PAD and dc == PAD)
            }
        )
        bcol = {v: j for j, v in enumerate(bias_vals)}
        bias_t = pool.tile([PR, len(bias_vals)], f32, tag="bias")
        for v, j in bcol.items():
            nc.gpsimd.memset(bias_t[:, j : j + 1], v)

        # center slot of kw zeroed so the flat norm folds skip it
        nc.gpsimd.memset(kw[:, 12 * SLW : 13 * SLW], 0.0)

        # ---- input DMAs: host pre-gathers SBUF layouts; per-s slices on
        # separate queues so the early pipeline blocks land in parallel ----
        S_ORDER = [2, 0, 1, 3, 4]
        ia_q = {2: nc.sync, 0: nc.scalar, 1: nc.scalar, 3: nc.sync, 4: nc.gpsimd}
        for s in S_ORDER:
            ia_q[s].dma_start(
                Ia[:, s * C * IW : (s + 1) * C * IW],
                i_ap[:, s * C * IW : (s + 1) * C * IW],
            )
        QSW = C * NQ * QB  # 3960 per s
        for s in S_ORDER:
            nc.scalar.dma_start(
                Qa[:, s * QSW : (s + 1) * QSW],
                q_ap[:, s * QSW : (s + 1) * QSW],
            )

        def sub_op(s, dc0, ndc):
            # d[(s,dc), (c,q), x] = Ia_s[(c,q), x+dc] - Ia_2[(c,q), x+2]
            nc.vector.tensor_sub(
                ap(d, (s * KS + dc0) * SLW, [[SLW, ndc], [XWO, C * NQ], [1, XWO]]),
                ap(Ia, s * C * IW + dc0, [[1, ndc], [XWI, C * NQ], [1, XWO]]),
                ap(Ia, 2 * C * IW + PAD, [[0, ndc], [XWI, C * NQ], [1, XWO]]),
            )

        # ---- per-s pipeline: sub -> square -> exp -> products -> fold ----
        first = True
        for si, s in enumerate(S_ORDER):
            if s == PAD:
                sub_op(s, 0, 2)
                sub_op(s, 3, 2)
                nc.scalar.activation(
                    kw[:, (s * KS) * SLW : (s * KS + 2) * SLW],
                    d[:, (s * KS) * SLW : (s * KS + 2) * SLW],
                    AF.Square,
                )
                nc.scalar.activation(
                    kw[:, (s * KS + 3) * SLW : (s * KS + 5) * SLW],
                    d[:, (s * KS + 3) * SLW : (s * KS + 5) * SLW],
                    AF.Square,
                )
            else:
                sub_op(s, 0, KS)
                nc.scalar.activation(
                    kw[:, (s * KS) * SLW : (s * KS + KS) * SLW],
                    d[:, (s * KS) * SLW : (s * KS + KS) * SLW],
                    AF.Square,
                )
            dcs = [0, 1, 3, 4] if s == PAD else list(range(KS))
            for dc in dcs:
                slot = s * KS + dc
                j = bcol[SHIFT + math.log(wy[s] * wy[dc])]
                nc.scalar.activation(
                    kw[:, slot * SLW : (slot + 1) * SLW],
                    kw[:, slot * SLW : (slot + 1) * SLW],
                    AF.Exp,
                    bias=bias_t[:, j : j + 1],
                    scale=-COEF,
                )

            def product(j, dc):
                slot = s * KS + dc
                nc.vector.tensor_mul(
                    ap(P5, j * CQN, [[NWX // NQ, C * NQ], [XWO, NCL], [1, XWO]]),
                    ap(kw, slot * SLW, [[XWO, C * NQ], [0, NCL], [1, XWO]]),
                    ap(Qa, s * QSW + dc, [[QB, C * NQ], [XWI, NCL], [1, XWO]]),
                )

            if si < 4:
                for j, dc in enumerate(dcs):
                    product(j, dc)
            else:
                # last group: interleave the norm tail + reciprocal between
                # products so rnorm is ready before the final division
                product(0, dcs[0])
                product(1, dcs[1])
                V = nc.vector
                V.tensor_add(
                    nB2[:, :], kw[:, 20 * SLW : 22 * SLW], kw[:, 22 * SLW : 24 * SLW]
                )
                V.tensor_add(nB2[:, :SLW], nB2[:, :SLW], nB2[:, SLW:])
                V.tensor_add(nB2[:, :SLW], nB2[:, :SLW], kw[:, 24 * SLW : 25 * SLW])
                V.tensor_add(norm[:, :], normA[:, :], nB2[:, :SLW])
                nc.scalar.activation(n32[:, :], norm[:, :], AF.Copy)
                product(2, dcs[2])
                nc.vector.reciprocal_approx_fast(r32[:, :], n32[:, :])
                product(3, dcs[3])
                nc.scalar.activation(rnh[:, :], r32[:, :], AF.Copy)
                product(4, dcs[4])

            # fold the group's 4-5 blocks into acc (big flat adds)
            nc.vector.tensor_add(
                s1[:, :], P5[:, : 2 * CQN], P5[:, 2 * CQN : 4 * CQN]
            )
            if first:
                nc.vector.tensor_add(acc[:, :], s1[:, :CQN], s1[:, CQN:])
                first = False
            else:
                nc.vector.tensor_add(s1[:, :CQN], s1[:, :CQN], s1[:, CQN:])
                nc.vector.tensor_add(
                    s1[:, CQN:], s1[:, :CQN], P5[:, 4 * CQN :]
                )
                nc.vector.tensor_add(acc[:, :], acc[:, :], s1[:, CQN:])

            if si == 3:
                # norm part A: fold the 20 finished slots (center zeroed)
                V = nc.vector
                V.tensor_add(
                    nA10[:, :], kw[:, : 10 * SLW], kw[:, 10 * SLW : 20 * SLW]
                )
                V.tensor_add(nA10[:, : 5 * SLW], nA10[:, : 5 * SLW], nA10[:, 5 * SLW :])
                V.tensor_add(
                    nA10[:, : 2 * SLW], nA10[:, : 2 * SLW], nA10[:, 2 * SLW : 4 * SLW]
                )
                V.tensor_add(normA[:, :], nA10[:, :SLW], nA10[:, SLW : 2 * SLW])
                V.tensor_add(normA[:, :], normA[:, :], nA10[:, 4 * SLW : 5 * SLW])

        # ---- final division per c, output DMAs spread over 3 queues ----
        out_q = [nc.sync, nc.scalar, nc.gpsimd]
        for c in range(C):
            nc.vector.tensor_mul(
                ap(ot, c * NWX, [[NCL * XWO, NQ], [XWO, NCL], [1, XWO]]),
                ap(acc, c * NWX, [[NCL * XWO, NQ], [XWO, NCL], [1, XWO]]),
                ap(rnh, c * NQ * XWO, [[XWO, NQ], [0, NCL], [1, XWO]]),
            )
            out_q[c].dma_start(
                out_ap[:, c * NWX : (c + 1) * NWX],
                ot[:, c * NWX : (c + 1) * NWX],
            )


def _build_program():
    import concourse.bacc as bacc
    import concourse.mybir as mybir
    from concourse import tile

    f16 = mybir.dt.float16

    nc = bacc.Bacc("TRN2", num_devices=8, debug=False)
    I_in = nc.dram_tensor("i_in", [PR, FW_IA], f16, kind="ExternalInput")
    Q_in = nc.dram_tensor("q_in", [PR, FW_QA], f16, kind="ExternalInput")
    OUT = nc.dram_tensor("out", [PR, CQN], f16, kind="ExternalOutput")

    with tile.TileContext(nc) as tc:
        _emit(tc, I_in.ap(), Q_in.ap(), OUT.ap())

    nc.compile()
    return nc


def _get_program():
    if "nc" not in _CACHE:
        _CACHE["nc"] = _build_program()
    return _CACHE["nc"]


def _gather_i(Xp_sl):
    """(C, 324, 84) padded slab -> (128, (s,c,q,xi44)) fp16."""
    t = np.stack([Xp_sl[:, s : s + H, :] for s in range(KS)])  # (s,C,320,84)
    t = t.reshape(KS, C, NQ, 64, 84)
    t = np.stack([t[..., 40 * xh : 40 * xh + XWI] for xh in range(2)])
    # (xh, s, c, q, rr, xi) -> (xh, rr, s, c, q, xi)
    t = t.transpose(0, 4, 1, 2, 3, 5)
    return np.ascontiguousarray(t.reshape(PR, FW_IA))


def _gather_q(Qp_sl):
    """(NCL, 324, 84) padded slab -> (128, (s,c,q,n,xi44)) fp16, c-replicated."""
    t = np.stack([Qp_sl[:, s : s + H, :] for s in range(KS)])  # (s,NCL,320,84)
    t = t.reshape(KS, NCL, NQ, 64, 84)
    t = np.stack([t[..., 40 * xh : 40 * xh + XWI] for xh in range(2)])
    # (xh, s, n, q, rr, xi) -> (xh, rr, s, q, n, xi)
    t = t.transpose(0, 4, 1, 3, 2, 5)  # (xh, rr, s, q, n, xi)
    t = t.reshape(2, 64, KS, 1, NQ, NCL, XWI)
    t = np.broadcast_to(t, (2, 64, KS, C, NQ, NCL, XWI))
    return np.ascontiguousarray(t.reshape(PR, FW_QA))


def _shard_inputs(Q, I):
    Qp = np.pad(
        np.asarray(Q, np.float32), ((0, 0), (0, 0), (PAD, PAD), (PAD, PAD))
    ).astype(np.float16)
    Ip = np.pad(
        np.asarray(I, np.float32), ((0, 0), (0, 0), (PAD, PAD), (PAD, PAD))
    ).astype(np.float16)
    in_maps = []
    for b in range(B):
        for xs in range(4):
            c0 = xs * XSL
            in_maps.append(
                {
                    "i_in": _gather_i(Ip[b, :, :, c0 : c0 + 84]),
                    "q_in": _gather_q(Qp[b, :, :, c0 : c0 + 84]),
                }
            )
    return in_maps


def _assemble(outs):
    # outs: 8 arrays (128, 3600 = (c,q,n,x)), core order = (b, xs)
    o = np.stack([np.asarray(x) for x in outs]).astype(np.float32)
    o = o.reshape(B, 4, 2, 64, C, NQ, NCL, XWO)
    # (b, xs, xh, rr, c, q, n, x) -> (b, c, n, row=(q,rr), col=(xs,xh,x))
    o = o.transpose(0, 4, 6, 5, 3, 1, 2, 7).reshape(B, C, NCL, H, W)
    return o


def run(Q, I, trace=False):
    from concourse.bass_utils import run_bass_kernel_spmd

    nc = _get_program()
    in_maps = _shard_inputs(Q, I)
    res = run_bass_kernel_spmd(nc, in_maps, list(range(8)), trace=trace)
    out = _assemble([res.results[i]["out"] for i in range(8)])
    return out, res


def kernel(Q, I):
    out, _ = run(Q, I)
    return out



# revision 2
# speedup vs baseline: 1.4631x; 1.4631x over previous
"""Bilateral filter (nn_BilateralFilter) Trainium2 Bass kernel.

Semantics (KERNEL_SIZE=5, THETA_ALPHA=2.0, THETA_BETA=0.1):
    w_k   = exp(-(dx^2+dy^2)/8)                      (24 offsets, center dropped)
    Ki    = exp(-50*(I(p+k) - I(p))^2)               per image channel c
    out[c,n,p] = sum_k w_k*Ki[c,k,p]*Q(n,p+k) / sum_k w_k*Ki[c,k,p]

Sharding: 8 cores = 2 batches x 4 col-slabs of 80 output cols.  Per core,
partitions = 128 = (xh in {0,1} col-half of 40) x (row mod 64); free dims =
(row-chunk q in [0,5), channel, x).  H=320 = 5*64 exactly, so every compute
op runs on 128 fully-occupied partitions.

v2: the 24-slot fold (sum_k of the P5 product blocks) moved off DVE onto the
Tensor engine: per product slot, 8 identity-stationary matmuls (bank-aligned
512-col chunks) accumulate P5 into a PSUM fp32 acc tile (start at slot 0,
stop at slot 23).  DVE now only does subs, the 24 products, the norm fold
tree and one final division op that reads acc straight from PSUM (fp32 1x)
with r32 broadcast over n.  Output is one contiguous [128,3600] DMA.
ACT does Square + per-slot Exp (bias = SHIFT + ln w_k; SHIFT=8 cancels in
the division) and the f16<->f32 reciprocal casts, fully overlapped.
"""

import math

import numpy as np

B, C, NCL = 2, 3, 6
H = W = 320
KS, PAD = 5, 2
SHIFT = 8.0
COEF = 50.0
XSL = W // 4              # 80 output cols per core slab
XWO = 40                  # output cols per half
XWI = XWO + 2 * PAD       # 44 input cols per half
NQ = 5                    # row chunks of 64
PR = 128
HP = H + 2 * PAD          # 324 padded rows

IW = NQ * XWI             # 220   Ia per (s,c)
FW_IA = KS * C * IW       # 3300
QB = NCL * XWI            # 264   Qa per (s,c,q)
FW_QA = KS * C * NQ * QB  # 19800 (Q replicated x3 over c)
SLW = C * NQ * XWO        # 600   d/kw per slot (c,q,x)
NWX = NQ * NCL * XWO      # 1200  per-c product block (q,n,x)
CQN = C * NWX             # 3600  per-j product block (c,q,n,x)
FW_D = KS * KS * SLW      # 15000
FW_N = C * NQ * XWO       # 600   norm (c,q,x)

# PSUM bank = 512 fp32 per partition; acc chunks must stay inside one bank
MM_CHUNKS = [(j * 512, min((j + 1) * 512, CQN)) for j in range((CQN + 511) // 512)]

_CACHE: dict = {}


def _emit(tc, i_ap, q_ap, out_ap):
    import concourse.bass as bass
    import concourse.mybir as mybir
    from concourse.masks import make_identity

    f16 = mybir.dt.float16
    f32 = mybir.dt.float32
    AF = mybir.ActivationFunctionType
    nc = tc.nc

    wy = [math.exp(-((s - PAD) ** 2) / 8.0) for s in range(KS)]

    def ap(t, off, dims):
        return bass.AP(tensor=t.tensor, offset=t.offset + off, ap=[[t.shape[1], PR]] + dims)

    with (
        tc.tile_pool(name="p", bufs=1) as pool,
        tc.tile_pool(name="p5p", bufs=3) as p5p,
        tc.tile_pool(name="ps", bufs=1, space="PSUM") as psp,
    ):
        Ia = pool.tile([PR, FW_IA], f16, tag="Ia")
        Qa = pool.tile([PR, FW_QA], f16, tag="Qa")
        d = pool.tile([PR, FW_D], f16, tag="d")
        kw = pool.tile([PR, FW_D], f16, tag="kw")
        ot = pool.tile([PR, CQN], f16, tag="out")

        nA10 = pool.tile([PR, 10 * SLW], f16, tag="nA10")
        normA = pool.tile([PR, FW_N], f16, tag="normA")
        nB2 = pool.tile([PR, 2 * SLW], f16, tag="nB2")
        norm = pool.tile([PR, FW_N], f16, tag="norm")
        n32 = pool.tile([PR, FW_N], f32, tag="n32")
        r32 = pool.tile([PR, FW_N], f32, tag="r32")

        ident = pool.tile([PR, PR], f16, tag="ident")
        acc = psp.tile([PR, CQN], f32, tag="acc")

        # per-slot exp biases SHIFT + ln(w_k) as const columns (5 distinct)
        bias_vals = sorted(
            {
                SHIFT + math.log(wy[s] * wy[dc])
                for s in range(KS)
                for dc in range(KS)
                if not (s == PAD and dc == PAD)
            }
        )
        bcol = {v: j for j, v in enumerate(bias_vals)}
        bias_t = pool.tile([PR, len(bias_vals)], f32, tag="bias")
        for v, j in bcol.items():
            nc.gpsimd.memset(bias_t[:, j : j + 1], v)

        # center slot of kw zeroed so the flat norm folds skip it
        nc.gpsimd.memset(kw[:, 12 * SLW : 13 * SLW], 0.0)

        make_identity(nc, ident[:, :])

        # ---- input DMAs: host pre-gathers SBUF layouts; per-s slices on
        # separate queues so the early pipeline blocks land in parallel ----
        S_ORDER = [2, 0, 1, 3, 4]
        ia_q = {2: nc.sync, 0: nc.scalar, 1: nc.scalar, 3: nc.sync, 4: nc.gpsimd}
        for s in S_ORDER:
            ia_q[s].dma_start(
                Ia[:, s * C * IW : (s + 1) * C * IW],
                i_ap[:, s * C * IW : (s + 1) * C * IW],
            )
        QSW = C * NQ * QB  # 3960 per s
        for s in S_ORDER:
            nc.scalar.dma_start(
                Qa[:, s * QSW : (s + 1) * QSW],
                q_ap[:, s * QSW : (s + 1) * QSW],
            )

        # PE p-state warmup: harmless closed-group matmuls into acc[:, :128]
        # during the input-DMA window (real chunk-0 group later resets with
        # start=True).  Keeps the Tensor engine clock ramping before the
        # product stream arrives.
        for _ in range(12):
            nc.tensor.matmul(
                acc[:, 0:PR], ident[:, :], ident[:, :], start=True, stop=True
            )

        def sub_op(s, dc0, ndc):
            # d[(s,dc), (c,q), x] = Ia_s[(c,q), x+dc] - Ia_2[(c,q), x+2]
            nc.vector.tensor_sub(
                ap(d, (s * KS + dc0) * SLW, [[SLW, ndc], [XWO, C * NQ], [1, XWO]]),
                ap(Ia, s * C * IW + dc0, [[1, ndc], [XWI, C * NQ], [1, XWO]]),
                ap(Ia, 2 * C * IW + PAD, [[0, ndc], [XWI, C * NQ], [1, XWO]]),
            )

        n_slots = 0

        def fold(p5t, slot_idx):
            # PE: accumulate this product block into PSUM acc (identity matmul)
            for c0, c1 in MM_CHUNKS:
                nc.tensor.matmul(
                    acc[:, c0:c1],
                    ident[:, :],
                    p5t[:, c0:c1],
                    start=(slot_idx == 0),
                    stop=(slot_idx == 23),
                )

        # ---- per-s pipeline: sub -> square -> exp -> products -> PE fold ----
        for si, s in enumerate(S_ORDER):
            if s == PAD:
                sub_op(s, 0, 2)
                sub_op(s, 3, 2)
                nc.scalar.activation(
                    kw[:, (s * KS) * SLW : (s * KS + 2) * SLW],
                    d[:, (s * KS) * SLW : (s * KS + 2) * SLW],
                    AF.Square,
                )
                nc.scalar.activation(
                    kw[:, (s * KS + 3) * SLW : (s * KS + 5) * SLW],
                    d[:, (s * KS + 3) * SLW : (s * KS + 5) * SLW],
                    AF.Square,
                )
            else:
                sub_op(s, 0, KS)
                nc.scalar.activation(
                    kw[:, (s * KS) * SLW : (s * KS + KS) * SLW],
                    d[:, (s * KS) * SLW : (s * KS + KS) * SLW],
                    AF.Square,
                )
            dcs = [0, 1, 3, 4] if s == PAD else list(range(KS))
            for dc in dcs:
                slot = s * KS + dc
                j = bcol[SHIFT + math.log(wy[s] * wy[dc])]
                nc.scalar.activation(
                    kw[:, slot * SLW : (slot + 1) * SLW],
                    kw[:, slot * SLW : (slot + 1) * SLW],
                    AF.Exp,
                    bias=bias_t[:, j : j + 1],
                    scale=-COEF,
                )

            def product(dc):
                slot = s * KS + dc
                p5t = p5p.tile([PR, CQN], f16, tag="p5")
                nc.vector.tensor_mul(
                    ap(p5t, 0, [[NWX // NQ, C * NQ], [XWO, NCL], [1, XWO]]),
                    ap(kw, slot * SLW, [[XWO, C * NQ], [0, NCL], [1, XWO]]),
                    ap(Qa, s * QSW + dc, [[QB, C * NQ], [XWI, NCL], [1, XWO]]),
                )
                return p5t

            if si < 4:
                for dc in dcs:
                    p5t = product(dc)
                    fold(p5t, n_slots)
                    n_slots += 1
            else:
                # last group: interleave the norm tail + reciprocal between
                # products so r32 is ready before the final division
                V = nc.vector
                for i, dc in enumerate(dcs):
                    p5t = product(dc)
                    fold(p5t, n_slots)
                    n_slots += 1
                    if i == 1:
                        V.tensor_add(
                            nB2[:, :],
                            kw[:, 20 * SLW : 22 * SLW],
                            kw[:, 22 * SLW : 24 * SLW],
                        )
                        V.tensor_add(nB2[:, :SLW], nB2[:, :SLW], nB2[:, SLW:])
                        V.tensor_add(
                            nB2[:, :SLW], nB2[:, :SLW], kw[:, 24 * SLW : 25 * SLW]
                        )
                        V.tensor_add(norm[:, :], normA[:, :], nB2[:, :SLW])
                        nc.scalar.activation(n32[:, :], norm[:, :], AF.Copy)
                    elif i == 2:
                        nc.vector.reciprocal_approx_fast(r32[:, :], n32[:, :])

            if si == 3:
                # norm part A: fold the 20 finished slots (center zeroed)
                V = nc.vector
                V.tensor_add(
                    nA10[:, :], kw[:, : 10 * SLW], kw[:, 10 * SLW : 20 * SLW]
                )
                V.tensor_add(nA10[:, : 5 * SLW], nA10[:, : 5 * SLW], nA10[:, 5 * SLW :])
                V.tensor_add(
                    nA10[:, : 2 * SLW], nA10[:, : 2 * SLW], nA10[:, 2 * SLW : 4 * SLW]
                )
                V.tensor_add(normA[:, :], nA10[:, :SLW], nA10[:, SLW : 2 * SLW])
                V.tensor_add(normA[:, :], normA[:, :], nA10[:, 4 * SLW : 5 * SLW])

        # ---- final division: one DVE op reading acc straight from PSUM ----
        CQ = C * NQ  # 15
        nc.vector.tensor_mul(
            ap(ot, 0, [[NCL * XWO, CQ], [XWO, NCL], [1, XWO]]),
            ap(acc, 0, [[NCL * XWO, CQ], [XWO, NCL], [1, XWO]]),
            ap(r32, 0, [[XWO, CQ], [0, NCL], [1, XWO]]),
        )
        nc.sync.dma_start(out_ap[:, :], ot[:, :])


def _build_program():
    import concourse.bacc as bacc
    import concourse.mybir as mybir
    from concourse import tile

    f16 = mybir.dt.float16

    nc = bacc.Bacc("TRN2", num_devices=8, debug=False)
    I_in = nc.dram_tensor("i_in", [PR, FW_IA], f16, kind="ExternalInput")
    Q_in = nc.dram_tensor("q_in", [PR, FW_QA], f16, kind="ExternalInput")
    OUT = nc.dram_tensor("out", [PR, CQN], f16, kind="ExternalOutput")

    with tile.TileContext(nc) as tc:
        _emit(tc, I_in.ap(), Q_in.ap(), OUT.ap())

    nc.compile()
    return nc


def _get_program():
    if "nc" not in _CACHE:
        _CACHE["nc"] = _build_program()
    return _CACHE["nc"]


def _gather_i(Xp_sl):
    """(C, 324, 84) padded slab -> (128, (s,c,q,xi44)) fp16."""
    t = np.stack([Xp_sl[:, s : s + H, :] for s in range(KS)])  # (s,C,320,84)
    t = t.reshape(KS, C, NQ, 64, 84)
    t = np.stack([t[..., 40 * xh : 40 * xh + XWI] for xh in range(2)])
    # (xh, s, c, q, rr, xi) -> (xh, rr, s, c, q, xi)
    t = t.transpose(0, 4, 1, 2, 3, 5)
    return np.ascontiguousarray(t.reshape(PR, FW_IA))


def _gather_q(Qp_sl):
    """(NCL, 324, 84) padded slab -> (128, (s,c,q,n,xi44)) fp16, c-replicated."""
    t = np.stack([Qp_sl[:, s : s + H, :] for s in range(KS)])  # (s,NCL,320,84)
    t = t.reshape(KS, NCL, NQ, 64, 84)
    t = np.stack([t[..., 40 * xh : 40 * xh + XWI] for xh in range(2)])
    # (xh, s, n, q, rr, xi) -> (xh, rr, s, q, n, xi)
    t = t.transpose(0, 4, 1, 3, 2, 5)  # (xh, rr, s, q, n, xi)
    t = t.reshape(2, 64, KS, 1, NQ, NCL, XWI)
    t = np.broadcast_to(t, (2, 64, KS, C, NQ, NCL, XWI))
    return np.ascontiguousarray(t.reshape(PR, FW_QA))


def _shard_inputs(Q, I):
    Qp = np.pad(
        np.asarray(Q, np.float32), ((0, 0), (0, 0), (PAD, PAD), (PAD, PAD))
    ).astype(np.float16)
    Ip = np.pad(
        np.asarray(I, np.float32), ((0, 0), (0, 0), (PAD, PAD), (PAD, PAD))
    ).astype(np.float16)
    in_maps = []
    for b in range(B):
        for xs in range(4):
            c0 = xs * XSL
            in_maps.append(
                {
                    "i_in": _gather_i(Ip[b, :, :, c0 : c0 + 84]),
                    "q_in": _gather_q(Qp[b, :, :, c0 : c0 + 84]),
                }
            )
    return in_maps


def _assemble(outs):
    # outs: 8 arrays (128, 3600 = (c,q,n,x)), core order = (b, xs)
    o = np.stack([np.asarray(x) for x in outs]).astype(np.float32)
    o = o.reshape(B, 4, 2, 64, C, NQ, NCL, XWO)
    # (b, xs, xh, rr, c, q, n, x) -> (b, c, n, row=(q,rr), col=(xs,xh,x))
    o = o.transpose(0, 4, 6, 5, 3, 1, 2, 7).reshape(B, C, NCL, H, W)
    return o


def run(Q, I, trace=False):
    from concourse.bass_utils import run_bass_kernel_spmd

    nc = _get_program()
    in_maps = _shard_inputs(Q, I)
    res = run_bass_kernel_spmd(nc, in_maps, list(range(8)), trace=trace)
    out = _assemble([res.results[i]["out"] for i in range(8)])
    return out, res


def kernel(Q, I):
    out, _ = run(Q, I)
    return out


# revision 4
# speedup vs baseline: 1.6096x; 1.1001x over previous
"""Bilateral filter (nn_BilateralFilter) Trainium2 Bass kernel.

Semantics (KERNEL_SIZE=5, THETA_ALPHA=2.0, THETA_BETA=0.1):
    w_k   = exp(-(dx^2+dy^2)/8)                      (24 offsets, center dropped)
    Ki    = exp(-50*(I(p+k) - I(p))^2)               per image channel c
    out[c,n,p] = sum_k w_k*Ki[c,k,p]*Q(n,p+k) / sum_k w_k*Ki[c,k,p]

Sharding: 8 cores = 2 batches x 4 col-slabs of 80 output cols.  Per core,
partitions = 128 = (xh in {0,1} col-half of 40) x (row mod 64); free dims =
(row-chunk q in [0,5), channel, x).

v3: every k-fold lives on the Tensor engine via identity-stationary matmuls
accumulating into PSUM:
  - acc  (sum_k kw*Q, 3600 fp32) in PSUM cols [0,3600), 512-col bank chunks
  - norm (sum_k kw) first 496 of its 600 cols in the PSUM bank-7 hole
    [3600,4096); the 104-col tail is a 6-op DVE tree (PSUM is 104 cols short)
DVE does subs, the 24 products, the tiny norm tail, reciprocal and the
final division (read straight from PSUM at 1x, split 1920+1680 so the first
output DMA overlaps the second division).  ACT does Square/Exp and the
fp32 stitch copies.  Outputs go to two contiguous DRAM tensors so the DMA
descriptors coalesce (a strided SBUF->HBM DMA is ~5x slower).
"""

import math

import numpy as np

B, C, NCL = 2, 3, 6
H = W = 320
KS, PAD = 5, 2
SHIFT = 8.0
COEF = 50.0
XSL = W // 4              # 80 output cols per core slab
XWO = 40                  # output cols per half
XWI = XWO + 2 * PAD       # 44 input cols per half
NQ = 5                    # row chunks of 64
PR = 128
HP = H + 2 * PAD          # 324 padded rows

IW = NQ * XWI             # 220   Ia per (s,c)
FW_IA = KS * C * IW       # 3300
QB = NCL * XWI            # 264   Qa per (s,c,q)
FW_QA = KS * C * NQ * QB  # 19800 (Q replicated x3 over c)
SLW = C * NQ * XWO        # 600   d/kw per slot (c,q,x)
NWX = NQ * NCL * XWO      # 1200  per-c product block (q,n,x)
CQN = C * NWX             # 3600  per-j product block (c,q,n,x)
FW_D = KS * KS * SLW      # 15000
FW_N = C * NQ * XWO       # 600   norm (c,q,x)

NPS = 496                 # norm cols accumulated in PSUM (bank-7 hole)
NTL = SLW - NPS           # 104   norm tail cols folded on DVE
CQ_A = 8                  # (c,q) blocks in the first div/DMA half
W_A = CQ_A * NCL * XWO    # 1920
W_B = CQN - W_A           # 1680

# PSUM bank = 512 fp32 per partition; acc chunks must stay inside one bank
MM_CHUNKS = [(j * 512, min((j + 1) * 512, CQN)) for j in range((CQN + 511) // 512)]

_CACHE: dict = {}


def _emit(tc, i_ap, q_ap, oa_ap, ob_ap):
    import concourse.bass as bass
    import concourse.mybir as mybir
    from concourse.masks import make_identity

    f16 = mybir.dt.float16
    f32 = mybir.dt.float32
    AF = mybir.ActivationFunctionType
    nc = tc.nc

    wy = [math.exp(-((s - PAD) ** 2) / 8.0) for s in range(KS)]

    def ap(t, off, dims):
        return bass.AP(tensor=t.tensor, offset=t.offset + off, ap=[[t.shape[1], PR]] + dims)

    with (
        tc.tile_pool(name="p", bufs=1) as pool,
        tc.tile_pool(name="p5p", bufs=4) as p5p,
        tc.tile_pool(name="ps", bufs=1, space="PSUM") as psp,
    ):
        Ia = pool.tile([PR, FW_IA], f16, tag="Ia")
        Qa = pool.tile([PR, FW_QA], f16, tag="Qa")
        d = pool.tile([PR, FW_D], f16, tag="d")
        kw = pool.tile([PR, FW_D], f16, tag="kw")
        ot = pool.tile([PR, CQN], f16, tag="out")

        normT = pool.tile([PR, 2 * NTL], f16, tag="normT")
        nt12 = pool.tile([PR, 12 * NTL], f16, tag="nt12")
        n32 = pool.tile([PR, FW_N], f32, tag="n32")
        r32 = pool.tile([PR, FW_N], f32, tag="r32")

        ident = pool.tile([PR, PR], f16, tag="ident")
        acc = psp.tile([PR, 4096], f32, tag="acc")

        # per-slot exp biases SHIFT + ln(w_k) as const columns (5 distinct)
        bias_vals = sorted(
            {
                SHIFT + math.log(wy[s] * wy[dc])
                for s in range(KS)
                for dc in range(KS)
                if not (s == PAD and dc == PAD)
            }
        )
        bcol = {v: j for j, v in enumerate(bias_vals)}
        bias_t = pool.tile([PR, len(bias_vals)], f32, tag="bias")
        for v, j in bcol.items():
            nc.gpsimd.memset(bias_t[:, j : j + 1], v)

        # center slot of kw zeroed so the norm folds can include it blindly
        nc.gpsimd.memset(kw[:, 12 * SLW : 13 * SLW], 0.0)

        make_identity(nc, ident[:, :])

        # ---- input DMAs: host pre-gathers SBUF layouts; per-s slices on
        # separate queues so the early pipeline blocks land in parallel ----
        S_ORDER = [2, 0, 1, 3, 4]
        ia_q = {2: nc.sync, 0: nc.scalar, 1: nc.scalar, 3: nc.sync, 4: nc.gpsimd}
        for s in S_ORDER:
            ia_q[s].dma_start(
                Ia[:, s * C * IW : (s + 1) * C * IW],
                i_ap[:, s * C * IW : (s + 1) * C * IW],
            )
        QSW = C * NQ * QB  # 3960 per s
        for s in S_ORDER:
            nc.scalar.dma_start(
                Qa[:, s * QSW : (s + 1) * QSW],
                q_ap[:, s * QSW : (s + 1) * QSW],
            )

        # PE p-state warmup during the input-DMA window (bank-0 region is
        # reset by the real chunk-0 group's start=True later).
        for _ in range(12):
            nc.tensor.matmul(
                acc[:, 0:PR], ident[:, :], ident[:, :], start=True, stop=True
            )

        def sub_op(s, dc0, ndc):
            # d[(s,dc), (c,q), x] = Ia_s[(c,q), x+dc] - Ia_2[(c,q), x+2]
            nc.vector.tensor_sub(
                ap(d, (s * KS + dc0) * SLW, [[SLW, ndc], [XWO, C * NQ], [1, XWO]]),
                ap(Ia, s * C * IW + dc0, [[1, ndc], [XWI, C * NQ], [1, XWO]]),
                ap(Ia, 2 * C * IW + PAD, [[0, ndc], [XWI, C * NQ], [1, XWO]]),
            )

        n_slots = 0   # fold-group index over the 24 non-center slots
        n_norm = 0    # norm-group index over all 25 slots

        # Bank 7 ([3584,4096): acc chunk 7 + norm region) is ONE accumulation
        # group: start=True zeroes a whole 2KB bank, so the first bank-7 write
        # (norm_mm of the first slot) starts it and the last fold chunk-7
        # stops it.  Other banks group per-chunk as usual.
        def fold(p5t, idx):
            for c0, c1 in MM_CHUNKS:
                in_b7 = c0 >= 3584
                nc.tensor.matmul(
                    acc[:, c0:c1],
                    ident[:, :],
                    p5t[:, c0:c1],
                    start=(idx == 0) and not in_b7,
                    stop=(idx == 23),
                )

        def norm_mm(slot, idx):
            nc.tensor.matmul(
                acc[:, 3600 : 3600 + NPS],
                ident[:, :],
                kw[:, slot * SLW : slot * SLW + NPS],
                start=(idx == 0),
                stop=False,
            )

        # ---- per-s pipeline: sub -> square -> exp(+norm mm) -> product ->
        #      PE fold ----
        for si, s in enumerate(S_ORDER):
            if s == PAD:
                sub_op(s, 0, 2)
                sub_op(s, 3, 2)
                nc.scalar.activation(
                    kw[:, (s * KS) * SLW : (s * KS + 2) * SLW],
                    d[:, (s * KS) * SLW : (s * KS + 2) * SLW],
                    AF.Square,
                )
                nc.scalar.activation(
                    kw[:, (s * KS + 3) * SLW : (s * KS + 5) * SLW],
                    d[:, (s * KS + 3) * SLW : (s * KS + 5) * SLW],
                    AF.Square,
                )
            else:
                sub_op(s, 0, KS)
                nc.scalar.activation(
                    kw[:, (s * KS) * SLW : (s * KS + KS) * SLW],
                    d[:, (s * KS) * SLW : (s * KS + KS) * SLW],
                    AF.Square,
                )
            for dc in range(KS):
                slot = s * KS + dc
                if slot != 12:
                    j = bcol[SHIFT + math.log(wy[s] * wy[dc])]
                    nc.scalar.activation(
                        kw[:, slot * SLW : (slot + 1) * SLW],
                        kw[:, slot * SLW : (slot + 1) * SLW],
                        AF.Exp,
                        bias=bias_t[:, j : j + 1],
                        scale=-COEF,
                    )
                norm_mm(slot, n_norm)
                n_norm += 1

            def product(dc):
                slot = s * KS + dc
                p5t = p5p.tile([PR, CQN], f16, tag="p5")
                nc.vector.tensor_mul(
                    ap(p5t, 0, [[NWX // NQ, C * NQ], [XWO, NCL], [1, XWO]]),
                    ap(kw, slot * SLW, [[XWO, C * NQ], [0, NCL], [1, XWO]]),
                    ap(Qa, s * QSW + dc, [[QB, C * NQ], [XWI, NCL], [1, XWO]]),
                )
                return p5t

            dcs = [0, 1, 3, 4] if s == PAD else list(range(KS))
            for i, dc in enumerate(dcs):
                p5t = product(dc)
                fold(p5t, n_slots)
                n_slots += 1
                if si == 4 and i == 1:
                    # norm 104-col tail: DVE tree over all 25 slots
                    V = nc.vector
                    V.tensor_add(
                        nt12[:, :],
                        ap(kw, NPS, [[SLW, 12], [1, NTL]]),
                        ap(kw, 12 * SLW + NPS, [[SLW, 12], [1, NTL]]),
                    )
                    V.tensor_add(
                        nt12[:, : 6 * NTL], nt12[:, : 6 * NTL], nt12[:, 6 * NTL :]
                    )
                    V.tensor_add(
                        nt12[:, : 3 * NTL],
                        nt12[:, : 3 * NTL],
                        nt12[:, 3 * NTL : 6 * NTL],
                    )
                    V.tensor_add(
                        normT[:, :NTL], nt12[:, :NTL], nt12[:, NTL : 2 * NTL]
                    )
                    V.tensor_add(
                        normT[:, NTL:], normT[:, :NTL], nt12[:, 2 * NTL : 3 * NTL]
                    )
                    V.tensor_add(
                        normT[:, :NTL],
                        normT[:, NTL:],
                        kw[:, 24 * SLW + NPS : 25 * SLW],
                    )
                    nc.scalar.activation(
                        n32[:, NPS:SLW], normT[:, :NTL], AF.Copy
                    )

        # ---- tail: drain norm PSUM, reciprocal, split division + DMAs ----
        nc.scalar.activation(n32[:, :NPS], acc[:, 3600 : 3600 + NPS], AF.Copy)
        nc.vector.reciprocal_approx_fast(r32[:, :], n32[:, :])

        CQ = C * NQ  # 15
        nc.vector.tensor_mul(
            ap(ot, 0, [[NCL * XWO, CQ_A], [XWO, NCL], [1, XWO]]),
            ap(acc, 0, [[NCL * XWO, CQ_A], [XWO, NCL], [1, XWO]]),
            ap(r32, 0, [[XWO, CQ_A], [0, NCL], [1, XWO]]),
        )
        nc.scalar.dma_start(oa_ap[:, :], ot[:, :W_A])
        nc.vector.tensor_mul(
            ap(ot, W_A, [[NCL * XWO, CQ - CQ_A], [XWO, NCL], [1, XWO]]),
            ap(acc, W_A, [[NCL * XWO, CQ - CQ_A], [XWO, NCL], [1, XWO]]),
            ap(r32, CQ_A * XWO, [[XWO, CQ - CQ_A], [0, NCL], [1, XWO]]),
        )
        nc.sync.dma_start(ob_ap[:, :], ot[:, W_A:])


def _build_program():
    import concourse.bacc as bacc
    import concourse.mybir as mybir
    from concourse import tile

    f16 = mybir.dt.float16

    nc = bacc.Bacc("TRN2", num_devices=8, debug=False)
    I_in = nc.dram_tensor("i_in", [PR, FW_IA], f16, kind="ExternalInput")
    Q_in = nc.dram_tensor("q_in", [PR, FW_QA], f16, kind="ExternalInput")
    OUT_A = nc.dram_tensor("out_a", [PR, W_A], f16, kind="ExternalOutput")
    OUT_B = nc.dram_tensor("out_b", [PR, W_B], f16, kind="ExternalOutput")

    with tile.TileContext(nc) as tc:
        _emit(tc, I_in.ap(), Q_in.ap(), OUT_A.ap(), OUT_B.ap())

    nc.compile()
    return nc


def _get_program():
    if "nc" not in _CACHE:
        _CACHE["nc"] = _build_program()
    return _CACHE["nc"]


def _gather_i(Xp_sl):
    """(C, 324, 84) padded slab -> (128, (s,c,q,xi44)) fp16."""
    t = np.stack([Xp_sl[:, s : s + H, :] for s in range(KS)])  # (s,C,320,84)
    t = t.reshape(KS, C, NQ, 64, 84)
    t = np.stack([t[..., 40 * xh : 40 * xh + XWI] for xh in range(2)])
    # (xh, s, c, q, rr, xi) -> (xh, rr, s, c, q, xi)
    t = t.transpose(0, 4, 1, 2, 3, 5)
    return np.ascontiguousarray(t.reshape(PR, FW_IA))


def _gather_q(Qp_sl):
    """(NCL, 324, 84) padded slab -> (128, (s,c,q,n,xi44)) fp16, c-replicated."""
    t = np.stack([Qp_sl[:, s : s + H, :] for s in range(KS)])  # (s,NCL,320,84)
    t = t.reshape(KS, NCL, NQ, 64, 84)
    t = np.stack([t[..., 40 * xh : 40 * xh + XWI] for xh in range(2)])
    # (xh, s, n, q, rr, xi) -> (xh, rr, s, q, n, xi)
    t = t.transpose(0, 4, 1, 3, 2, 5)  # (xh, rr, s, q, n, xi)
    t = t.reshape(2, 64, KS, 1, NQ, NCL, XWI)
    t = np.broadcast_to(t, (2, 64, KS, C, NQ, NCL, XWI))
    return np.ascontiguousarray(t.reshape(PR, FW_QA))


def _shard_inputs(Q, I):
    Qp = np.pad(
        np.asarray(Q, np.float32), ((0, 0), (0, 0), (PAD, PAD), (PAD, PAD))
    ).astype(np.float16)
    Ip = np.pad(
        np.asarray(I, np.float32), ((0, 0), (0, 0), (PAD, PAD), (PAD, PAD))
    ).astype(np.float16)
    in_maps = []
    for b in range(B):
        for xs in range(4):
            c0 = xs * XSL
            in_maps.append(
                {
                    "i_in": _gather_i(Ip[b, :, :, c0 : c0 + 84]),
                    "q_in": _gather_q(Qp[b, :, :, c0 : c0 + 84]),
                }
            )
    return in_maps


def _assemble(outs):
    # outs: 8 arrays (128, 3600 = (c,q,n,x)), core order = (b, xs)
    o = np.stack([np.asarray(x) for x in outs]).astype(np.float32)
    o = o.reshape(B, 4, 2, 64, C, NQ, NCL, XWO)
    # (b, xs, xh, rr, c, q, n, x) -> (b, c, n, row=(q,rr), col=(xs,xh,x))
    o = o.transpose(0, 4, 6, 5, 3, 1, 2, 7).reshape(B, C, NCL, H, W)
    return o


def run(Q, I, trace=False):
    from concourse.bass_utils import run_bass_kernel_spmd

    nc = _get_program()
    in_maps = _shard_inputs(Q, I)
    res = run_bass_kernel_spmd(nc, in_maps, list(range(8)), trace=trace)
    out = _assemble(
        [
            np.concatenate(
                [res.results[i]["out_a"], res.results[i]["out_b"]], axis=1
            )
            for i in range(8)
        ]
    )
    return out, res


def kernel(Q, I):
    out, _ = run(Q, I)
    return out
